# revision 8
# baseline (speedup 1.0000x reference)
"""Two-layer GraphSAGE (mean aggr) on 8 Trainium2 NeuronCores.

Strategy (graph/data parallel, dst-sharded):
  - Nodes are partitioned into 8 contiguous shards (6250 per core); each core
    owns all edges whose dst lands in its shard (~100K edges/core).
  - Features table x (and later h) is replicated in each core's HBM; per-edge
    source rows are fetched with dma_gather (512B rows, full HBM bandwidth).
  - Mean aggregation is computed on the PE: for each 256-dst window, edges are
    processed in 128-edge chunks; a weighted one-hot [128e, 256d] built on the
    DVE (fused is_equal*inv_deg tensor_scalar) is the moving operand against
    the gathered chunk [128e, 128f] as stationary -> PSUM accumulates
    meanT [128f, 256d] directly (float32r, 1 cyc/row).
  - Layer matmuls W_l/W_r accumulate into the same PSUM group; bias+ReLU fused
    on the scalar engine.  h shards are transposed back to row-major, written
    to HBM, and exchanged with an in-kernel AllGather; layer 2 repeats the
    pipeline (identical edge schedule) and ends with a transposed log_softmax.
  - Gather indices are int16 (hardware requirement), so the node table is kept
    as two <=32768-row halves and every gather piece reads from one half.
  - SPMD: all 8 cores run one program; the per-(window,half) chunk counts are
    maxed across cores on the host, with padding chunks (weight 0, index 0).
"""

import math
import numpy as np
from dataclasses import dataclass, field

# ---------------------------------------------------------------- constants
N_NODES = 50000
IN_CH = 128
HIDDEN = 128
OUT_CH = 40
N_EDGES = 800000
N_CORES = 8

P = 128            # partitions / chunk size (edges per chunk)
WIN = 256          # dst nodes per aggregation window (PSUM moving width)
PIECE_MAX = 8      # max chunks per dma_gather (SWDGE ring holds 1024 descriptors)


@dataclass
class Cfg:
    n: int = N_NODES
    e: int = N_EDGES
    f: int = IN_CH          # feature width (== HIDDEN)
    out: int = OUT_CH
    cores: int = N_CORES
    piece_max: int = PIECE_MAX

    @property
    def nshard(self):
        assert self.n % self.cores == 0
        return self.n // self.cores

    @property
    def half(self):
        assert self.n % 2 == 0
        h = self.n // 2
        assert h <= 32768
        return h

    @property
    def nw(self):
        return math.ceil(self.nshard / WIN)

    @property
    def n_node_tiles(self):
        return math.ceil(self.nshard / P)


@dataclass
class Chunk:
    w: int
    half: int
    start: bool = False
    stop: bool = False


@dataclass
class Piece:
    half: int
    c0: int
    nchunks: int


@dataclass
class Plan:
    cfg: Cfg
    chunks: list = field(default_factory=list)
    pieces: list = field(default_factory=list)
    per_core: list = field(default_factory=list)   # per-core input dict
    common: dict = field(default_factory=dict)     # shared input arrays

    @property
    def nchunk(self):
        return len(self.chunks)


def _make_plan(cfg: Cfg, x, edge_index, weights):
    src = np.asarray(edge_index[0], dtype=np.int64)
    dst = np.asarray(edge_index[1], dtype=np.int64)
    x = np.ascontiguousarray(np.asarray(x, dtype=np.float32))

    deg = np.bincount(dst, minlength=cfg.n)
    inv_cnt = (1.0 / np.maximum(deg, 1)).astype(np.float32)

    core = dst // cfg.nshard
    reld = dst - core * cfg.nshard
    w = reld // WIN
    half = (src >= cfg.half).astype(np.int64)

    nw = cfg.nw
    key = (core * nw + w) * 2 + half
    cnt = np.bincount(key, minlength=cfg.cores * nw * 2).reshape(cfg.cores, nw, 2)
    S = np.ceil(cnt / P).astype(np.int64).max(axis=0)          # [nw, 2]
    # every window must have at least one chunk so PSUM gets initialized
    need = S.sum(axis=1) == 0
    S[need, 0] = 1

    # ---- chunk sequence: window pairs, halves grouped within the pair
    plan = Plan(cfg=cfg)
    group_order = []     # (w, half) in emission order
    for p0 in range(0, nw, 2):
        ws = [p0] + ([p0 + 1] if p0 + 1 < nw else [])
        for h in (0, 1):
            for w_ in ws:
                if S[w_, h]:
                    group_order.append((w_, h))
                    for _ in range(int(S[w_, h])):
                        plan.chunks.append(Chunk(w=w_, half=h))
    # start/stop flags
    first_seen, last_seen = {}, {}
    for i, ch in enumerate(plan.chunks):
        first_seen.setdefault(ch.w, i)
        last_seen[ch.w] = i
    for w_, i in first_seen.items():
        plan.chunks[i].start = True
    for w_, i in last_seen.items():
        plan.chunks[i].stop = True

    # ---- pieces: maximal same-half runs, capped at piece_max
    i = 0
    while i < len(plan.chunks):
        j = i
        while (j < len(plan.chunks) and plan.chunks[j].half == plan.chunks[i].half
               and j - i < cfg.piece_max):
            j += 1
        plan.pieces.append(Piece(half=plan.chunks[i].half, c0=i, nchunks=j - i))
        i = j

    nchunk = plan.nchunk
    e_pad = nchunk * P

    # ---- per-core edge arrays
    order = np.lexsort((half, w, core))   # groups of (core, w, half), contiguous
    src_s, w_s, half_s, core_s, reld_s, dst_s = (
        src[order], w[order], half[order], core[order], reld[order], dst[order])

    # slice boundaries per (core, w, half)
    ends = np.cumsum(cnt.ravel()).reshape(cfg.cores, nw, 2)
    starts = ends - cnt

    for k in range(cfg.cores):
        idxv = np.zeros(e_pad, dtype=np.int16)
        drv = np.full(e_pad, -1.0, dtype=np.float32)
        wvv = np.zeros(e_pad, dtype=np.float32)
        pos = 0
        for (w_, h) in group_order:
            a, b = int(starts[k, w_, h]), int(ends[k, w_, h])
            m = b - a
            slots = int(S[w_, h]) * P
            if m:
                idxv[pos:pos + m] = (src_s[a:b] - h * cfg.half).astype(np.int16)
                drv[pos:pos + m] = (reld_s[a:b] - w_ * WIN).astype(np.float32)
                wvv[pos:pos + m] = inv_cnt[dst_s[a:b]]
            pos += slots
        assert pos == e_pad
        # wrap layout for dma_gather: index i -> [i%16, i//16], replicated x8
        idx_w = np.tile(idxv.reshape(e_pad // 16, 16).T, (8, 1))
        per = {
            "gidx": np.ascontiguousarray(idx_w),
            "dstrel": np.ascontiguousarray(drv.reshape(nchunk, P).T),
            "wv": np.ascontiguousarray(wvv.reshape(nchunk, P).T),
            "x_own": np.ascontiguousarray(
                x[k * cfg.nshard:(k + 1) * cfg.nshard]).astype(
                    __import__("ml_dtypes").bfloat16),
        }
        plan.per_core.append(per)

    W_l1, b_l1, W_r1, W_l2, b_l2, W_r2 = weights

    def bf16(a):
        import ml_dtypes
        return np.ascontiguousarray(np.asarray(a, np.float32).astype(ml_dtypes.bfloat16))

    plan.common = {
        "x_full": bf16(x),
        "iota": np.tile(np.arange(WIN, dtype=np.float32), (P, 1)),
        "ident": np.eye(P, dtype=np.float32),
        "identb": bf16(np.eye(P, dtype=np.float32)),
        "wl1": bf16(W_l1),
        "wr1": bf16(W_r1),
        "bl1": np.asarray(b_l1, np.float32).reshape(-1, 1),
        "wl2": bf16(W_l2),
        "wr2": bf16(W_r2),
        "bl2": np.asarray(b_l2, np.float32).reshape(-1, 1),
    }
    return plan


# ------------------------------------------------------------------ builder
def _build(plan: Plan):
    from concourse import bass, bacc, tile, mybir

    cfg = plan.cfg
    f32 = mybir.dt.float32
    bf16 = mybir.dt.bfloat16
    i16 = mybir.dt.int16
    Alu = mybir.AluOpType
    Act = mybir.ActivationFunctionType
    F = cfg.f
    OUT = cfg.out
    nw = cfg.nw
    nshard = cfg.nshard
    nchunk = plan.nchunk
    e_pad = nchunk * P

    nc = bacc.Bacc("TRN2", target_bir_lowering=False, debug=False,
                   num_devices=cfg.cores)

    x_full = nc.dram_tensor("x_full", [cfg.n, F], bf16, kind="ExternalInput")
    x_own = nc.dram_tensor("x_own", [nshard, F], bf16, kind="ExternalInput")
    gidx = nc.dram_tensor("gidx", [P, e_pad // 16], i16, kind="ExternalInput")
    dstrel = nc.dram_tensor("dstrel", [P, nchunk], f32, kind="ExternalInput")
    wv = nc.dram_tensor("wv", [P, nchunk], f32, kind="ExternalInput")
    iota = nc.dram_tensor("iota", [P, WIN], f32, kind="ExternalInput")
    ident = nc.dram_tensor("ident", [P, P], f32, kind="ExternalInput")
    identb = nc.dram_tensor("identb", [P, P], bf16, kind="ExternalInput")
    wl1 = nc.dram_tensor("wl1", [F, F], bf16, kind="ExternalInput")
    wr1 = nc.dram_tensor("wr1", [F, F], bf16, kind="ExternalInput")
    bl1 = nc.dram_tensor("bl1", [F, 1], f32, kind="ExternalInput")
    wl2 = nc.dram_tensor("wl2", [F, OUT], bf16, kind="ExternalInput")
    wr2 = nc.dram_tensor("wr2", [F, OUT], bf16, kind="ExternalInput")
    bl2 = nc.dram_tensor("bl2", [OUT, 1], f32, kind="ExternalInput")
    outd = nc.dram_tensor("out", [nshard, OUT], f32, kind="ExternalOutput")

    with tile.TileContext(nc) as tc:
        with (
            tc.tile_pool(name="const", bufs=1) as constp,
            tc.tile_pool(name="gath", bufs=2) as gathp,
            tc.tile_pool(name="oh", bufs=4) as ohp,
            tc.tile_pool(name="mean", bufs=3) as meanp,
            tc.tile_pool(name="stage", bufs=3) as stagep,
            tc.tile_pool(name="ls", bufs=4) as lsp,
            tc.tile_pool(name="dram", bufs=1, space="DRAM") as dramp,
            tc.tile_pool(name="psA", bufs=4, space="PSUM") as psA,
            tc.tile_pool(name="psB", bufs=2, space="PSUM") as psB,
            tc.tile_pool(name="psT", bufs=2, space="PSUM") as psT,
        ):
            # ---------------- constants and metadata
            def load(pool, dram_t, shape, dtype, tag):
                t = pool.tile(shape, dtype, tag=tag, name=tag)
                nc.sync.dma_start(out=t[:], in_=dram_t.ap())
                return t

            iota_t = load(constp, iota, [P, WIN], f32, "iota_t")
            ident_t = load(constp, ident, [P, P], f32, "ident_t")
            identb_t = load(constp, identb, [P, P], bf16, "identb_t")
            wl1_t = load(constp, wl1, [F, F], bf16, "wl1_t")
            wr1_t = load(constp, wr1, [F, F], bf16, "wr1_t")
            bl1_t = load(constp, bl1, [F, 1], f32, "bl1_t")
            wl2_t = load(constp, wl2, [F, OUT], bf16, "wl2_t")
            wr2_t = load(constp, wr2, [F, OUT], bf16, "wr2_t")
            bl2_t = load(constp, bl2, [OUT, 1], f32, "bl2_t")
            gidx_t = load(constp, gidx, [P, e_pad // 16], i16, "gidx_t")
            dstrel_t = load(constp, dstrel, [P, nchunk], f32, "dstrel_t")
            wv_t = load(constp, wv, [P, nchunk], f32, "wv_t")

            xT = constp.tile([P, nw * WIN], bf16, tag="xT", name="xT")
            hT = constp.tile([P, nw * WIN], bf16, tag="hT", name="hT")

            h_bounce = dramp.tile([nshard, F], bf16, name="h_bounce")
            h_full = dramp.tile([cfg.n, F], bf16, name="h_full")

            # zero the node-tail columns so later reads stay finite
            tail0 = cfg.n_node_tiles * P
            if tail0 < nw * WIN:
                nc.vector.memset(xT[:, tail0:], 0.0)

            # ---------------- transpose own x -> xT [F, nodes]
            for t in range(cfg.n_node_tiles):
                rows = min(P, nshard - t * P)
                xs = stagep.tile([P, F], bf16, tag="xs", name="xs")
                nc.sync.dma_start(out=xs[:rows, :], in_=x_own.ap()[t * P:t * P + rows, :])
                tp = psT.tile([P, P], bf16, tag="tp", name="tp")
                nc.tensor.transpose(tp[:], xs[:], identb_t[:])
                nc.vector.tensor_copy(out=xT[:, t * P:(t + 1) * P], in_=tp[:])

            # ---------------- per-layer pipeline
            def finish_window_l1(w):
                pt = live.pop(w)
                mean_t = meanp.tile([P, WIN], bf16, tag="mean_t", name="mean_t")
                nc.scalar.activation(mean_t[:], pt[:], Act.Copy)
                hp = psB.tile([P, WIN], f32, tag="hp", name="hp")
                nc.tensor.matmul(hp[:], lhsT=wl1_t[:],
                                 rhs=mean_t[:], start=True, stop=False)
                nc.tensor.matmul(hp[:], lhsT=wr1_t[:],
                                 rhs=xT[:, w * WIN:(w + 1) * WIN],
                                 start=False, stop=True)
                nc.scalar.activation(hT[:, w * WIN:(w + 1) * WIN], hp[:],
                                     Act.Relu, bias=bl1_t[:, 0:1], scale=1.0)
                for s in range(WIN // P):
                    col0 = w * WIN + s * P
                    if col0 >= nshard:
                        break
                    rows = min(P, nshard - col0)
                    tp = psT.tile([P, P], bf16, tag="tp", name="tp")
                    nc.tensor.transpose(tp[:], hT[:, col0:col0 + P], identb_t[:])
                    hr = stagep.tile([P, F], bf16, tag="hr", name="hr")
                    nc.vector.tensor_copy(out=hr[:], in_=tp[:])
                    nc.sync.dma_start(out=h_bounce[col0:col0 + rows, :],
                                      in_=hr[:rows, :])

            def finish_window_l2(w):
                pt = live.pop(w)
                mean_t = meanp.tile([P, WIN], bf16, tag="mean_t", name="mean_t")
                nc.scalar.activation(mean_t[:], pt[:], Act.Copy)
                lp = psB.tile([P, WIN], f32, tag="hp", name="hp")
                nc.tensor.matmul(lp[:OUT, :], lhsT=wl2_t[:],
                                 rhs=mean_t[:], start=True, stop=False)
                nc.tensor.matmul(lp[:OUT, :], lhsT=wr2_t[:],
                                 rhs=hT[:, w * WIN:(w + 1) * WIN],
                                 start=False, stop=True)
                lgt = stagep.tile([P, WIN], f32, tag="lgt", name="lgt")
                nc.vector.tensor_scalar(out=lgt[:OUT, :], in0=lp[:OUT, :],
                                        scalar1=bl2_t[:, 0:1], scalar2=None,
                                        op0=Alu.add)
                for s in range(WIN // P):
                    col0 = w * WIN + s * P
                    if col0 >= nshard:
                        break
                    rows = min(P, nshard - col0)
                    tp2 = psT.tile([P, P], f32, tag="tp", name="tp")
                    nc.tensor.transpose(tp2[:, :OUT], lgt[:OUT, s * P:(s + 1) * P],
                                        ident_t[:OUT, :OUT])
                    mx = lsp.tile([P, 1], f32, tag="mx", name="mx")
                    nc.vector.tensor_reduce(mx[:], tp2[:, :OUT],
                                            axis=mybir.AxisListType.X, op=Alu.max)
                    t1 = lsp.tile([P, OUT], f32, tag="t1", name="t1")
                    nc.vector.tensor_scalar(out=t1[:], in0=tp2[:, :OUT],
                                            scalar1=mx[:, 0:1], scalar2=None,
                                            op0=Alu.subtract)
                    e1 = lsp.tile([P, OUT], f32, tag="e1", name="e1")
                    nc.scalar.activation(e1[:], t1[:], Act.Exp)
                    s1 = lsp.tile([P, 1], f32, tag="s1", name="s1")
                    nc.vector.tensor_reduce(s1[:], e1[:],
                                            axis=mybir.AxisListType.X, op=Alu.add)
                    ls_ = lsp.tile([P, 1], f32, tag="ls_", name="ls_")
                    nc.scalar.activation(ls_[:], s1[:], Act.Ln)
                    ot = lsp.tile([P, OUT], f32, tag="ot", name="ot")
                    nc.vector.tensor_scalar(out=ot[:], in0=t1[:],
                                            scalar1=ls_[:, 0:1], scalar2=None,
                                            op0=Alu.subtract)
                    nc.sync.dma_start(out=outd.ap()[col0:col0 + rows, :],
                                      in_=ot[:rows, :])

            def do_layer(layer):
                if layer == 1:
                    tabs = (x_full.ap()[0:cfg.half, :], x_full.ap()[cfg.half:cfg.n, :])
                    fin = finish_window_l1
                else:
                    tabs = (h_full[0:cfg.half, :], h_full[cfg.half:cfg.n, :])
                    fin = finish_window_l2
                for pc_i, piece in enumerate(plan.pieces):
                    pcn = piece.nchunks
                    g = gathp.tile([P, cfg.piece_max, F], bf16, tag="g", name="g")
                    nc.gpsimd.dma_gather(
                        out_ap=g[:, :pcn, :],
                        in_ap=tabs[piece.half],
                        idxs_ap=gidx_t[:, piece.c0 * 8:(piece.c0 + pcn) * 8],
                        num_idxs=pcn * P,
                        num_idxs_reg=pcn * P,
                        elem_size=F,
                    )
                    for j in range(pcn):
                        ci = piece.c0 + j
                        ch = plan.chunks[ci]
                        oh = ohp.tile([P, WIN], bf16, tag="oh", name="oh")
                        nc.vector.tensor_scalar(
                            out=oh[:], in0=iota_t[:],
                            scalar1=dstrel_t[:, ci:ci + 1],
                            scalar2=wv_t[:, ci:ci + 1],
                            op0=Alu.is_equal, op1=Alu.mult)
                        if ch.start:
                            live[ch.w] = psA.tile([P, WIN], f32, tag="aggps",
                                                  name="aggps")
                        nc.tensor.matmul(live[ch.w][:],
                                         lhsT=g[:, j, :],
                                         rhs=oh[:],
                                         start=ch.start, stop=ch.stop)
                        if ch.stop:
                            fin(ch.w)

            live = {}
            do_layer(1)
            assert not live
            nc.gpsimd.collective_compute(
                "AllGather", Alu.bypass,
                replica_groups=[list(range(cfg.cores))],
                ins=[h_bounce.opt()],
                outs=[h_full.opt()],
            )
            do_layer(2)
            assert not live

    nc.compile()
    return nc


# ------------------------------------------------------------------ runner
_CACHE = {}


def _get_compiled(cfg, x, edge_index, weights):
    key = hash((np.asarray(edge_index)[:, ::997].tobytes(),
                np.asarray(edge_index).shape))
    hit = _CACHE.get(key)
    if hit is None:
        plan = _make_plan(cfg, x, edge_index, weights)
        nc = _build(plan)
        _CACHE[key] = (plan, nc)
        hit = (plan, nc)
    return hit


_LAST_EXEC_NS = None


def kernel(x, edge_index, W_l1, b_l1, W_r1, W_l2, b_l2, W_r2, trace=False):
    global _LAST_EXEC_NS
    from concourse.bass_utils import run_bass_kernel_spmd

    cfg = Cfg()
    plan, nc = _get_compiled(cfg, x, edge_index,
                             (W_l1, b_l1, W_r1, W_l2, b_l2, W_r2))
    in_maps = []
    for k in range(cfg.cores):
        m = dict(plan.common)
        m.update(plan.per_core[k])
        in_maps.append(m)
    res = run_bass_kernel_spmd(nc, in_maps, core_ids=list(range(cfg.cores)),
                               trace=trace)
    _LAST_EXEC_NS = res.exec_time_ns
    out = np.concatenate([res.results[k]["out"] for k in range(cfg.cores)], axis=0)
    return out.astype(np.float32)
